# revision 1
# baseline (speedup 1.0000x reference)
"""CGCNNConv Trainium2 kernel: 8-core edge-parallel (dst-sorted) implementation.

Math:
  z = [atom[dst] | atom[src] | edge_feat]           [E, 192]
  y_c = z @ W_c.T + b_c ; y_f = z @ W_f.T + b_f     [E, 64] each
  BN over edge axis (training stats, biased var), then
  msg = sigmoid(BN(y_f)) * softplus(BN(y_c))
  out = atom + segment_sum(msg, dst)

Device strategy per core (cores own disjoint 6272-atom ranges; edges sorted
by dst, routed to the owner of dst; 128-edge tiles grouped per 128-atom
scatter window; identical program on all 8 cores, per-core data):
  - Prologue: P_src = atom @ W[:,64:128].T table [50304,128] fp16 in HBM
    (rows >= 50000 are zero -> padding sentinel); P_loc = local-window
    dst-projection [6272,128] kept in SBUF.
  - Pass 1 (chunked): y = onehot_dst matmul(P_loc) + EF^T matmul(W3|b)
    [PSUM] + indirect-gather(P_src[src]); y and y^2 stored interleaved
    fp16, y to HBM; per-channel sum(y), sum(y^2) via ones-column matmuls
    accumulated in PSUM.
  - Stats AllReduce [1,256] across 8 cores; BN scale/shift derived on-chip
    (rsqrt = exp(-0.5 ln)); filter half sign-flipped so the gate input is
    -x_f; broadcast to [128,256] via rank-1 matmul.
  - Pass 2 (chunked): affine (chunk-wide DVE, fp16 2x); ACT ops batched per
    chunk-pair to minimize LUT-set reloads: Exp (both halves), Ln(1+x)
    (softplus for core half, ln(1+e^-xf) for filter), Exp(-x) (gate);
    msg product; segment-sum matmul lhsT=onehot_em accumulated per
    128-atom group in PSUM; add atom rows; DMA out.
"""

import os
import sys

import numpy as np

for _p in ("/opt/trn_rl_repo", os.path.expanduser("~/.axon_site/_ro/trn_rl_repo")):
    if os.path.isdir(_p) and _p not in sys.path:
        sys.path.insert(0, _p)

N_ATOMS = 50000
N_EDGES = 800000
D = 64          # node/edge feature dim
C = 128         # packed channels: 0:64 core, 64:128 filter
N_CORES = 8
GA = 128                       # atoms per scatter group
G_PER_CORE = 49
A_PER_CORE = G_PER_CORE * GA   # 6272 (128-aligned; 8*6272 = 50176 >= 50000)
A_PAD = A_PER_CORE
TILE = 128
CHUNK = 16                     # tiles per DMA/compute chunk
QCH = 8                        # tiles per PSUM accumulation chunk
ATOM_PAD = 50304               # 393*128 table rows (>= 50000 are zero)
PAD_ROW = 50200                # sentinel zero row for padded edges
BN_EPS = 1e-5

LAST_EXEC_NS = None


# --------------------------------------------------------------------------
# Host-side preprocessing
# --------------------------------------------------------------------------

def _preprocess(atom_features, edge_features, edge_indices):
    src = np.asarray(edge_indices[:, 0], dtype=np.int64)
    dst = np.asarray(edge_indices[:, 1], dtype=np.int64)
    order = np.argsort(dst, kind="stable")
    dst_s = dst[order]

    group_edges = np.zeros((N_CORES, G_PER_CORE + 1), dtype=np.int64)
    for c in range(N_CORES):
        lo = c * A_PER_CORE
        gb = [lo + g * GA for g in range(G_PER_CORE)] + [lo + A_PER_CORE]
        group_edges[c] = np.searchsorted(dst_s, np.array(gb), side="left")

    cnt = group_edges[:, 1:] - group_edges[:, :-1]
    T_g = np.maximum(1, (cnt + TILE - 1) // TILE).max(axis=0)
    NT = int(T_g.sum())
    n_chunks = (NT + CHUNK - 1) // CHUNK
    T_g[-1] += n_chunks * CHUNK - NT
    NT = n_chunks * CHUNK
    L = NT * TILE
    t_starts = np.concatenate([[0], np.cumsum(T_g)])[:-1]

    per_core = []
    for c in range(N_CORES):
        ids = np.full(L, -1, dtype=np.int64)
        for g in range(G_PER_CORE):
            e0, e1 = group_edges[c, g], group_edges[c, g + 1]
            s = t_starts[g] * TILE
            ids[s:s + (e1 - e0)] = order[e0:e1]
        valid = ids >= 0
        idc = np.where(valid, ids, 0)

        ef = np.where(valid[:, None], edge_features[idc], 0.0).astype(np.float32)
        onesr = valid.astype(np.float32)       # 0 on padding -> no bias term
        ef_T = np.concatenate([ef.T, onesr[None, :]], axis=0).astype(np.float16)

        srcv = np.where(valid, src[idc], PAD_ROW).astype(np.int32)
        gidx = np.repeat(np.arange(G_PER_CORE), np.asarray(T_g) * TILE)
        base = c * A_PER_CORE + gidx * GA
        rel = np.where(valid, dst[idc] - base, -1).astype(np.int32)

        rel3 = rel.reshape(NT, TILE)                       # [t, e]
        ar = np.arange(TILE)
        oh = rel3[:, :, None] == ar[None, None, :]         # [t, e, a]
        oh_am = np.ascontiguousarray(
            oh.transpose(2, 0, 1).reshape(TILE, L)).astype(np.float16)
        oh_em = np.ascontiguousarray(
            oh.transpose(1, 0, 2).reshape(TILE, L)).astype(np.float16)

        a0 = c * A_PER_CORE
        arows = np.zeros((A_PAD, D), np.float32)
        n = max(0, min(A_PAD, N_ATOMS - a0))
        arows[:n] = atom_features[a0:a0 + n]
        atl = np.zeros((D, A_PER_CORE), np.float16)
        atl[:, :n] = atom_features[a0:a0 + n].T

        per_core.append({
            "ef_T": np.ascontiguousarray(ef_T),
            "idx_src": np.ascontiguousarray(srcv.reshape(NT, TILE).T),
            "oh_am": oh_am,
            "oh_em": oh_em,
            "atom_rows": arows,
            "atom_T_loc": atl,
        })

    return per_core, list(map(int, T_g)), list(map(int, t_starts)), NT, n_chunks


# --------------------------------------------------------------------------
# Device program
# --------------------------------------------------------------------------

def _build_nc(NT, T_g, t_starts, n_chunks, repeat=1, sim_mode=False):
    import concourse.bacc as bacc
    import concourse.mybir as mybir
    import concourse.tile as tile
    from concourse.bass import AP, IndirectOffsetOnAxis

    f32 = mybir.dt.float32
    f16 = mybir.dt.float16
    i32 = mybir.dt.int32
    ADD = mybir.AluOpType.add
    MUL = mybir.AluOpType.mult
    SUB = mybir.AluOpType.subtract
    AF = mybir.ActivationFunctionType

    def rep_mid(ap2d, times):
        # [P, F] -> [P, times, F] with middle step 0 (repeat along chunk)
        return AP(ap2d.tensor, ap2d.offset, [ap2d.ap[0], [0, times], ap2d.ap[1]])

    L = NT * TILE
    nc = bacc.Bacc(None)

    ef_T = nc.dram_tensor("ef_T", [D + 1, L], f16, kind="ExternalInput")
    idx_src = nc.dram_tensor("idx_src", [TILE, NT], i32, kind="ExternalInput")
    oh_am_d = nc.dram_tensor("oh_am", [TILE, L], f16, kind="ExternalInput")
    oh_em_d = nc.dram_tensor("oh_em", [TILE, L], f16, kind="ExternalInput")
    atom_rows = nc.dram_tensor("atom_rows", [A_PAD, D], f32, kind="ExternalInput")
    atom_T_loc = nc.dram_tensor("atom_T_loc", [D, A_PER_CORE], f16,
                                kind="ExternalInput")
    atom_T = nc.dram_tensor("atom_T", [D, ATOM_PAD], f16, kind="ExternalInput")
    w1T = nc.dram_tensor("w1T", [D, C], f16, kind="ExternalInput")
    w2T = nc.dram_tensor("w2T", [D, C], f16, kind="ExternalInput")
    w3b = nc.dram_tensor("w3b", [D + 1, C], f16, kind="ExternalInput")
    ones_d = nc.dram_tensor("ones", [1, C], f32, kind="ExternalInput")
    onescol_d = nc.dram_tensor("ones_col", [TILE, 1], f16, kind="ExternalInput")
    gb_d = nc.dram_tensor("gb", [1, 2 * C], f32, kind="ExternalInput")
    out_d = nc.dram_tensor("out", [A_PAD, D], f32, kind="ExternalOutput")

    P_src = nc.dram_tensor("P_src", [ATOM_PAD, C], f16)
    y_dram = nc.dram_tensor("y_dram", [TILE, L], f16)
    stats_in = nc.dram_tensor("stats_in", [1, 2 * C], f32)
    stats_out = nc.dram_tensor("stats_out", [1, 2 * C], f32, addr_space="Shared")

    core_ids = list(range(N_CORES))

    g_of_t = []
    for g, tg in enumerate(T_g):
        g_of_t += [g] * tg

    with tile.TileContext(nc) as tc:
        with (
            tc.tile_pool(name="const", bufs=1) as const_p,
            tc.tile_pool(name="efp", bufs=2) as ef_p,
            tc.tile_pool(name="gat", bufs=2) as gat_p,
            tc.tile_pool(name="ohp", bufs=2) as oh_p,
            tc.tile_pool(name="ych", bufs=2) as y_p,
            tc.tile_pool(name="act", bufs=2) as act_p,
            tc.tile_pool(name="small", bufs=4) as small_p,
            tc.tile_pool(name="qps", bufs=2, space="PSUM") as qps_p,
            tc.tile_pool(name="sps", bufs=1, space="PSUM") as sps_p,
            tc.tile_pool(name="segps", bufs=2, space="PSUM") as seg_p,
            tc.tile_pool(name="ppch", bufs=2) as pp_p,
        ):
            # ---------- resident constants ----------
            w1T_sb = const_p.tile([D, C], f16)
            nc.sync.dma_start(out=w1T_sb[:], in_=w1T[:])
            w2T_sb = const_p.tile([D, C], f16)
            nc.sync.dma_start(out=w2T_sb[:], in_=w2T[:])
            w3b_sb = const_p.tile([D + 1, C], f16)
            nc.sync.dma_start(out=w3b_sb[:], in_=w3b[:])
            ones_sb = const_p.tile([1, C], f32)
            nc.sync.dma_start(out=ones_sb[:], in_=ones_d[:])
            onescol_sb = const_p.tile([TILE, 1], f16)
            nc.sync.dma_start(out=onescol_sb[:], in_=onescol_d[:])
            gb_sb = const_p.tile([1, 2 * C], f32)
            nc.sync.dma_start(out=gb_sb[:], in_=gb_d[:])
            idxs_sb = const_p.tile([TILE, NT], i32)
            nc.sync.dma_start(out=idxs_sb[:], in_=idx_src[:])

            for _rep in range(repeat):
                # ---------- prologue: P_src table + local P_loc ----------
                ACH = 2048
                a_done = 0
                while a_done < ATOM_PAD:
                    an = min(ACH, ATOM_PAD - a_done)
                    ntile = an // TILE
                    at_ch = ef_p.tile([D, ACH], f16, tag="atch")
                    nc.sync.dma_start(out=at_ch[:, :an],
                                      in_=atom_T[:, a_done:a_done + an])
                    pp_ch = pp_p.tile([TILE, (ACH // TILE) * C], f16)
                    for j4 in range(0, ntile, 4):
                        jn = min(4, ntile - j4)
                        pps = qps_p.tile([TILE, 4 * C], f32, space="PSUM",
                                         tag="q")
                        for j in range(j4, j4 + jn):
                            nc.tensor.matmul(
                                pps[:, (j - j4) * C:(j - j4 + 1) * C],
                                lhsT=at_ch[:, j * TILE:(j + 1) * TILE],
                                rhs=w2T_sb[:], start=True, stop=True)
                        nc.scalar.copy(
                            out=pp_ch[:, j4 * C:(j4 + jn) * C],
                            in_=pps[:, 0:jn * C])
                    view = P_src[a_done:a_done + an, :].rearrange(
                        "(j p) c -> p j c", p=TILE)
                    pp3 = pp_ch[:].rearrange("p (j c) -> p j c", c=C)
                    nc.sync.dma_start(out=view, in_=pp3[:, :ntile, :])
                    a_done += an

                ploc_sb = const_p.tile([TILE, G_PER_CORE * C], f16, tag="ploc")
                for gb0 in range(0, G_PER_CORE, 16):
                    gn = min(16, G_PER_CORE - gb0)
                    atl_ch = ef_p.tile([D, ACH], f16, tag="atch", name="atl_ch")
                    nc.sync.dma_start(
                        out=atl_ch[:, :gn * TILE],
                        in_=atom_T_loc[:, gb0 * TILE:(gb0 + gn) * TILE])
                    for g in range(gb0, gb0 + gn):
                        pps = qps_p.tile([TILE, C], f32, space="PSUM", tag="q")
                        nc.tensor.matmul(
                            pps[:],
                            lhsT=atl_ch[:, (g - gb0) * TILE:(g - gb0 + 1) * TILE],
                            rhs=w1T_sb[:], start=True, stop=True)
                        nc.scalar.copy(out=ploc_sb[:, g * C:(g + 1) * C],
                                       in_=pps[:])

                # ---------- pass 1 ----------
                stats_ps = sps_p.tile([1, 2 * C], f32, space="PSUM")

                for ch in range(n_chunks):
                    c0 = ch * CHUNK
                    ef_ch = ef_p.tile([D + 1, CHUNK * TILE], f16, tag="efch")
                    nc.sync.dma_start(
                        out=ef_ch[:], in_=ef_T[:, c0 * TILE:(c0 + CHUNK) * TILE])
                    oham_ch = oh_p.tile([TILE, CHUNK * TILE], f16, tag="oham")
                    nc.sync.dma_start(
                        out=oham_ch[:],
                        in_=oh_am_d[:, c0 * TILE:(c0 + CHUNK) * TILE])
                    gat = gat_p.tile([TILE, CHUNK * C], f16)
                    for j in range(CHUNK):
                        t = c0 + j
                        nc.gpsimd.indirect_dma_start(
                            out=gat[:, j * C:(j + 1) * C], out_offset=None,
                            in_=P_src[:],
                            in_offset=IndirectOffsetOnAxis(
                                ap=idxs_sb[:, t:t + 1], axis=0))

                    pair = y_p.tile([TILE, CHUNK * 2 * C], f16, tag="pair")
                    for q0 in range(0, CHUNK, QCH):
                        qp = qps_p.tile([TILE, QCH * C], f32, space="PSUM",
                                        tag="q")
                        for j in range(q0, q0 + QCH):
                            sl = qp[:, (j - q0) * C:(j - q0 + 1) * C]
                            nc.tensor.matmul(
                                sl, lhsT=ef_ch[:, j * TILE:(j + 1) * TILE],
                                rhs=w3b_sb[:], start=True, stop=False)
                            g = g_of_t[c0 + j]
                            nc.tensor.matmul(
                                sl, lhsT=oham_ch[:, j * TILE:(j + 1) * TILE],
                                rhs=ploc_sb[:, g * C:(g + 1) * C],
                                start=False, stop=True)
                        nc.vector.tensor_tensor(
                            out=pair[:, q0 * C:(q0 + QCH) * C],
                            in0=qp[:], in1=gat[:, q0 * C:(q0 + QCH) * C], op=ADD)
                    nc.vector.tensor_tensor(
                        out=pair[:, CHUNK * C:], in0=pair[:, 0:CHUNK * C],
                        in1=pair[:, 0:CHUNK * C], op=MUL)
                    for j in range(CHUNK):
                        t = c0 + j
                        rhs_ap = AP(pair[:].tensor, pair[:].offset + j * C,
                                    [pair[:].ap[0], [CHUNK * C, 2], [1, C]])
                        nc.tensor.matmul(
                            stats_ps[:], lhsT=onescol_sb[:], rhs=rhs_ap,
                            start=(t == 0), stop=(t == NT - 1))
                    nc.sync.dma_start(
                        out=y_dram[:, c0 * TILE:(c0 + CHUNK) * TILE],
                        in_=pair[:, 0:CHUNK * C])

                # ---------- BN stats all-reduce + params ----------
                st_sb = small_p.tile([1, 2 * C], f32, tag="st")
                nc.vector.tensor_copy(out=st_sb[:], in_=stats_ps[:])
                nc.sync.dma_start(out=stats_in[:], in_=st_sb[:])
                if sim_mode:
                    nc.sync.dma_start(out=stats_out[:], in_=stats_in[:])
                else:
                    nc.gpsimd.collective_compute(
                        "AllReduce", ADD,
                        replica_groups=[core_ids],
                        ins=[stats_in[:]],
                        outs=[stats_out[:]],
                    )
                stg = small_p.tile([1, 2 * C], f32, tag="stg")
                nc.sync.dma_start(out=stg[:], in_=stats_out[:])

                bn = small_p.tile([1, 5 * C], f32, tag="bn")
                mu = bn[:, 0:C]
                m2 = bn[:, C:2 * C]
                var = bn[:, 2 * C:3 * C]
                sd = bn[:, 3 * C:4 * C]
                inv = bn[:, 4 * C:5 * C]
                inv_e = 1.0 / float(N_EDGES)
                nc.vector.tensor_scalar_mul(mu, stg[:, 0:C], inv_e)
                nc.vector.tensor_scalar_mul(m2, stg[:, C:2 * C], inv_e)
                nc.vector.tensor_tensor(out=var, in0=mu, in1=mu, op=MUL)
                nc.vector.tensor_tensor(out=var, in0=m2, in1=var, op=SUB)
                nc.vector.tensor_scalar_add(var, var, BN_EPS)
                nc.scalar.activation(sd, var, AF.Ln)
                nc.scalar.activation(inv, sd, AF.Exp, scale=-0.5)
                ab = small_p.tile([1, 2 * C], f32, tag="ab")
                nc.vector.tensor_tensor(out=ab[:, 0:C], in0=inv,
                                        in1=gb_sb[:, 0:C], op=MUL)
                tmp = small_p.tile([1, C], f32, tag="tmp")
                nc.vector.tensor_tensor(out=tmp[:], in0=mu, in1=ab[:, 0:C],
                                        op=MUL)
                nc.vector.tensor_tensor(out=ab[:, C:2 * C],
                                        in0=gb_sb[:, C:2 * C], in1=tmp[:],
                                        op=SUB)
                # sign-flip the filter half so the gate input is -x_f
                nc.vector.tensor_scalar_mul(ab[:, D:C], ab[:, D:C], -1.0)
                nc.vector.tensor_scalar_mul(ab[:, C + D:2 * C],
                                            ab[:, C + D:2 * C], -1.0)
                abps = qps_p.tile([TILE, 2 * C], f32, space="PSUM", tag="q")
                nc.tensor.matmul(abps[:], lhsT=ones_sb[:], rhs=ab[:],
                                 start=True, stop=True)
                ab_bc = const_p.tile([TILE, 2 * C], f16, tag="abbc")
                nc.vector.tensor_copy(out=ab_bc[:], in_=abps[:])

                # ---------- pass 2 ----------
                seg_holder = {}

                def affine(ch):
                    c0 = ch * CHUNK
                    yc = y_p.tile([TILE, CHUNK * TILE], f16, tag="y2ch")
                    nc.sync.dma_start(
                        out=yc[:],
                        in_=y_dram[:, c0 * TILE:(c0 + CHUNK) * TILE])
                    ohem_ch = oh_p.tile([TILE, CHUNK * TILE], f16, tag="ohem")
                    nc.sync.dma_start(
                        out=ohem_ch[:],
                        in_=oh_em_d[:, c0 * TILE:(c0 + CHUNK) * TILE])
                    yc3 = yc[:].rearrange("p (j c) -> p j c", c=C)
                    yn = act_p.tile([TILE, CHUNK * TILE], f16, tag="yn")
                    yn3 = yn[:].rearrange("p (j c) -> p j c", c=C)
                    nc.vector.tensor_tensor(
                        out=yn3, in0=yc3, in1=rep_mid(ab_bc[:, 0:C], CHUNK),
                        op=MUL)
                    nc.vector.tensor_tensor(
                        out=yn3, in0=yn3, in1=rep_mid(ab_bc[:, C:2 * C], CHUNK),
                        op=ADD)
                    return yn, ohem_ch

                for chp in range(0, n_chunks, 2):
                    chs = [c for c in (chp, chp + 1) if c < n_chunks]
                    prep = [affine(c) for c in chs]
                    eg = [act_p.tile([TILE, CHUNK * TILE], f16, tag="eg", name=f"eg{k}")
                          for k in range(len(chs))]
                    for k, (yn, _) in enumerate(prep):
                        nc.scalar.activation(eg[k][:], yn[:], AF.Exp)
                    for k in range(len(chs)):
                        nc.scalar.activation(eg[k][:], eg[k][:], AF.Ln,
                                             bias=1.0)
                    gt = [act_p.tile([TILE, CHUNK * D], f16, tag="gt", name=f"gt{k}")
                          for k in range(len(chs))]
                    for k in range(len(chs)):
                        eg3 = eg[k][:].rearrange("p (j c) -> p j c", c=C)
                        gt3 = gt[k][:].rearrange("p (j c) -> p j c", c=D)
                        nc.scalar.activation(gt3, eg3[:, :, D:C], AF.Exp,
                                             scale=-1.0)
                    for k, ch in enumerate(chs):
                        c0 = ch * CHUNK
                        eg3 = eg[k][:].rearrange("p (j c) -> p j c", c=C)
                        gt3 = gt[k][:].rearrange("p (j c) -> p j c", c=D)
                        msg = small_p.tile([TILE, CHUNK * D], f16, tag="msg")
                        msg3 = msg[:].rearrange("p (j c) -> p j c", c=D)
                        nc.vector.tensor_tensor(
                            out=msg3, in0=eg3[:, :, 0:D], in1=gt3, op=MUL)
                        ohem_ch = prep[k][1]
                        for j in range(CHUNK):
                            t = c0 + j
                            g = g_of_t[t]
                            first = (t == t_starts[g])
                            last = (t == t_starts[g] + T_g[g] - 1)
                            if first:
                                seg_holder[g] = seg_p.tile(
                                    [TILE, D], f32, space="PSUM", tag="seg",
                                    name=f"seg{g}")
                            cur_ps = seg_holder[g]
                            nc.tensor.matmul(
                                cur_ps[:],
                                lhsT=ohem_ch[:, j * TILE:(j + 1) * TILE],
                                rhs=msg[:, j * D:(j + 1) * D],
                                start=first, stop=last)
                            if last:
                                at = small_p.tile([TILE, D], f32, tag="at")
                                nc.sync.dma_start(
                                    out=at[:],
                                    in_=atom_rows[g * GA:(g + 1) * GA, :])
                                ot = small_p.tile([TILE, D], f32, tag="ot")
                                nc.vector.tensor_tensor(
                                    out=ot[:], in0=cur_ps[:], in1=at[:],
                                    op=ADD)
                                nc.sync.dma_start(
                                    out=out_d[g * GA:(g + 1) * GA, :],
                                    in_=ot[:])

    nc.finalize()
    return nc


# --------------------------------------------------------------------------
# Entry point
# --------------------------------------------------------------------------

def kernel(atom_features, edge_features, W_filter, b_filter, gamma_filter,
           beta_filter, W_core, b_core, gamma_core, beta_core, edge_indices):
    global LAST_EXEC_NS
    from concourse.bass_utils import run_bass_kernel_spmd

    atom_features = np.asarray(atom_features, np.float32)
    edge_features = np.asarray(edge_features, np.float32)

    per_core, T_g, t_starts, NT, n_chunks = _preprocess(
        atom_features, edge_features, np.asarray(edge_indices))

    W_all = np.vstack([np.asarray(W_core, np.float32),
                       np.asarray(W_filter, np.float32)])
    b_all = np.concatenate([np.asarray(b_core, np.float32),
                            np.asarray(b_filter, np.float32)])
    gamma_all = np.concatenate([np.asarray(gamma_core, np.float32),
                                np.asarray(gamma_filter, np.float32)])
    beta_all = np.concatenate([np.asarray(beta_core, np.float32),
                               np.asarray(beta_filter, np.float32)])

    atom_T = np.zeros((D, ATOM_PAD), np.float16)
    atom_T[:, :N_ATOMS] = atom_features.T
    w1T = np.ascontiguousarray(W_all[:, 0:D].T).astype(np.float16)
    w2T = np.ascontiguousarray(W_all[:, D:2 * D].T).astype(np.float16)
    w3b = np.concatenate([W_all[:, 2 * D:3 * D].T, b_all[None, :]],
                         axis=0).astype(np.float16)
    gb = np.concatenate([gamma_all, beta_all])[None, :].astype(np.float32)

    shared = {
        "atom_T": atom_T,
        "w1T": w1T,
        "w2T": w2T,
        "w3b": np.ascontiguousarray(w3b),
        "ones": np.ones((1, C), np.float32),
        "ones_col": np.ones((TILE, 1), np.float16),
        "gb": gb,
    }
    in_maps = []
    for c in range(N_CORES):
        m = dict(shared)
        m.update(per_core[c])
        in_maps.append(m)

    nc = _build_nc(NT, T_g, t_starts, n_chunks)

    trace = bool(int(os.environ.get("KERNEL_TRACE", "0")))
    res = run_bass_kernel_spmd(nc, in_maps, list(range(N_CORES)), trace=trace)
    LAST_EXEC_NS = res.exec_time_ns

    out = np.zeros((N_ATOMS, D), np.float32)
    for c in range(N_CORES):
        n = min(A_PER_CORE, N_ATOMS - c * A_PER_CORE)
        out[c * A_PER_CORE:c * A_PER_CORE + n] = res.results[c]["out"][:n]
    return out

